# revision 1
# baseline (speedup 1.0000x reference)
"""Trainium2 Bass kernel: 16-head MHA (B=4, S=2048, E=1024, Dh=64), 8 cores.

Sharding: core c handles batch b = c//2 and head-group g = c%2 (8 heads).
Each core computes its 8 heads' attention plus the partial output
projection in transposed layout oT[e, s]; the host sums the two
head-group partials per batch, transposes, and adds bo.

Per-core dataflow (all matmuls bf16 with fp32 PSUM accumulation):
  qT/kT[d, s]  = Wq/Wk.T @ xT          (per head-pair, d stacked 2x64)
  v[t, hd]     = xT.T @ Wv + ones.T@bv (natural layout, + ones col for colsum)
  scoresT[t,s] = kT.T @ qT   (row-tiled: 2 heads in rows 0-63 / 64-127)
  expT         = exp(0.125 * scoresT)  (ScalarE, cast to bf16)
  zT_un[d,s],colsum[s] = v_aug.T @ expT  (M=65: row 64 = colsum)
  zT           = zT_un * bcast(1/colsum) (bcast via K=1 matmul)
  oT[e, s]    += Wo_h.T @ zT_h  (accumulated over the core's 8 heads)
"""

import numpy as np
import ml_dtypes

B, S, E = 4, 2048, 1024
H, Dh = 16, 64
N_CORES = 8
HPC = 8          # heads per core
MP = 4           # head-pairs per core
SC, SCW = 4, 512  # s-chunks
TC, TCW = 16, 128  # t-chunks
KE = 8           # k-tiles over E
ECN = 8          # e-chunks of 128 (outT partition tiles)

BF16 = ml_dtypes.bfloat16

_PROG = None


def _build_program(repeats=None, timing=False, parts=3):
    """Emit the Bass/Tile program. Returns (nc, names_dict).

    repeats: if set, wrap the whole body in a For_i loop (for marginal
    per-iteration HW timing; not used by the graded kernel() path).
    timing: demote the real output to internal DRAM and expose a tiny
    dummy output instead, so timing calls don't pay output transfers.
    parts: 1 = projections only, 2 = + attention, 3 = full (default).
    """
    from contextlib import ExitStack

    import concourse.mybir as mybir
    import concourse.tile as tile
    from concourse import bacc

    dt = mybir.dt
    AF = mybir.ActivationFunctionType
    OP = mybir.AluOpType

    nc = bacc.Bacc(None, target_bir_lowering=False, debug=False)
    with tile.TileContext(nc) as tc:
        with tc.tile_pool(name="dram", bufs=1, space="DRAM") as dram:
            xT_d = dram.tile([E, S], dt.bfloat16, kind="ExternalInput")
            wq_d = dram.tile([E, HPC * Dh], dt.bfloat16, kind="ExternalInput")
            wk_d = dram.tile([E, HPC * Dh], dt.bfloat16, kind="ExternalInput")
            wv_d = dram.tile([E, HPC * Dh], dt.bfloat16, kind="ExternalInput")
            wo_d = dram.tile([128, MP, E], dt.bfloat16, kind="ExternalInput")
            bq_d = dram.tile([128, MP], dt.float32, kind="ExternalInput")
            bk_d = dram.tile([128, MP], dt.float32, kind="ExternalInput")
            bv_d = dram.tile([1, HPC * Dh], dt.bfloat16, kind="ExternalInput")
            if timing:
                oT_d = dram.tile([E, S], dt.float32, kind="Internal")
                dummy_d = dram.tile([1, 4], dt.bfloat16, kind="ExternalOutput")
            else:
                oT_d = dram.tile([E, S], dt.float32, kind="ExternalOutput")
                dummy_d = None

            with (
                tc.tile_pool(name="const", bufs=1) as const,
                tc.tile_pool(name="work", bufs=2) as work,
                tc.tile_pool(name="norm1", bufs=1) as norm1,
                tc.tile_pool(name="zpool", bufs=1) as zpool,
                tc.tile_pool(name="psum_sT", bufs=2, space="PSUM") as psum_sT,
                tc.tile_pool(name="psum_z", bufs=4, space="PSUM") as psum_z,
                ExitStack() as _es,
            ):
                if repeats is not None:
                    _es.enter_context(tc.For_i(
                        0, repeats, 1,
                        hint_engines=(
                            mybir.EngineType.PE, mybir.EngineType.Activation,
                            mybir.EngineType.DVE, mybir.EngineType.SP,
                            mybir.EngineType.Pool,
                        ),
                    ))
                # ---- persistent SBUF ----
                xT = const.tile([128, KE, S], dt.bfloat16)
                wq = const.tile([128, KE, HPC * Dh], dt.bfloat16)
                wk = const.tile([128, KE, HPC * Dh], dt.bfloat16)
                wv = const.tile([128, KE, HPC * Dh], dt.bfloat16)
                wo = const.tile([128, MP, E], dt.bfloat16)
                bqk = const.tile([128, 2 * MP], dt.float32)
                onesbv = const.tile([1, HPC * Dh + 128], dt.bfloat16)
                qT2 = const.tile([128, MP, S], dt.bfloat16)
                kT2 = const.tile([128, MP, S], dt.bfloat16)
                v_sb = const.tile([128, TC, HPC, Dh + 1], dt.bfloat16)

                # ---- input DMAs ----
                xTr = xT_d[:].rearrange("(a p) c -> p a c", p=128)
                nc.sync.dma_start(wk[:, :, :], wk_d[:].rearrange("(a p) c -> p a c", p=128))
                nc.sync.dma_start(bqk[:, 0:MP], bq_d[:])
                nc.sync.dma_start(bqk[:, MP:2 * MP], bk_d[:])
                for k in range(KE):
                    nc.sync.dma_start(xT[:, k:k + 1, :], xTr[:, k:k + 1, :])
                nc.sync.dma_start(wv[:, :, :], wv_d[:].rearrange("(a p) c -> p a c", p=128))
                nc.sync.dma_start(wq[:, :, :], wq_d[:].rearrange("(a p) c -> p a c", p=128))
                nc.sync.dma_start(wo[:, :, :], wo_d[:])
                nc.sync.dma_start(onesbv[0:1, 0:HPC * Dh], bv_d[:])
                nc.vector.memset(onesbv[0:1, HPC * Dh:], 1.0)
                nc.vector.memset(v_sb[:, :, :, Dh:Dh + 1], 1.0)
                if dummy_d is not None:
                    nc.sync.dma_start(dummy_d[:, :], onesbv[0:1, 0:4])

                # ---- projections: kT2 (layout [d(2 heads), s]) + v, all s ----
                def proj_qk(w_sb, boff, dst, m, sc):
                    ssl = slice(sc * SCW, (sc + 1) * SCW)
                    p = psum_z.tile([128, SCW], dt.float32, tag="z")
                    for k in range(KE):
                        nc.tensor.matmul(
                            p[:, :],
                            w_sb[:, k, m * 128:(m + 1) * 128],
                            xT[:, k, ssl],
                            start=(k == 0), stop=(k == KE - 1),
                        )
                    nc.vector.tensor_scalar_add(
                        dst[:, m, ssl], p[:, :], bqk[:, boff + m:boff + m + 1]
                    )

                for m in range(MP):
                    for sc in range(SC):
                        proj_qk(wk, MP, kT2, m, sc)

                # v projection (natural layout [t, hd] + bias + ones col)
                for t in range(TC):
                    tsl = slice(t * TCW, (t + 1) * TCW)
                    p = psum_z.tile([128, HPC * Dh], dt.float32, tag="z")
                    for k in range(KE):
                        nc.tensor.matmul(
                            p[:, :], xT[:, k, tsl], wv[:, k, :],
                            start=(k == 0), stop=False,
                        )
                    nc.tensor.matmul(
                        p[:, :], onesbv[0:1, HPC * Dh:HPC * Dh + 128],
                        onesbv[0:1, 0:HPC * Dh], start=False, stop=True,
                    )
                    nc.vector.tensor_copy(
                        v_sb[:, t, :, 0:Dh],
                        p[:, :].rearrange("p (h c) -> p h c", c=Dh),
                    )

                # ---- attention + output projection, per s-chunk ----
                def emit_norm(hh, pz, zT2, m):
                    """Normalize head (pair m, parity hh) into zT2[:, m].
                    Odd heads land on partitions 64-127 via a shift DMA."""
                    cs = norm1.tile([1, SCW], dt.bfloat16, tag="cs")
                    nc.vector.tensor_copy(cs[0:1, :], pz[Dh:Dh + 1, :])
                    pbc = psum_z.tile([Dh, SCW], dt.float32, tag="z")
                    nc.tensor.matmul(
                        pbc[:, :], onesbv[0:1, HPC * Dh:HPC * Dh + Dh], cs[0:1, :],
                        start=True, stop=True,
                    )
                    bch = norm1.tile([Dh, SCW], dt.float32, tag="bch")
                    nc.vector.reciprocal(bch[:, :], pbc[:, :])
                    if hh == 0:
                        nc.vector.tensor_tensor(
                            zT2[0:Dh, m, :], pz[0:Dh, :], bch[:, :], OP.mult
                        )
                    else:
                        ztmp = work.tile([Dh, SCW], dt.bfloat16, tag="ztmp")
                        nc.vector.tensor_tensor(
                            ztmp[:, :], pz[0:Dh, :], bch[:, :], OP.mult
                        )
                        nc.sync.dma_start(zT2[Dh:2 * Dh, m, :], ztmp[:, :])

                def outproj_thunks(sc, zT2):
                    """One thunk per e-chunk group: 4 K=128 matmuls + evict."""
                    ssl = slice(sc * SCW, (sc + 1) * SCW)
                    def mk(ec):
                        def thunk():
                            po = psum_z.tile([128, SCW], dt.float32, tag="z",
                                             name=f"po_{sc}_{ec}")
                            for m in range(MP):
                                nc.tensor.matmul(
                                    po[:, :],
                                    wo[:, m, ec * 128:(ec + 1) * 128],
                                    zT2[:, m, :],
                                    start=(m == 0), stop=(m == MP - 1),
                                )
                            ob = work.tile([128, SCW], dt.float32, tag="ob",
                                           name=f"ob_{sc}_{ec}")
                            nc.vector.tensor_copy(ob[:, :], po[:, :])
                            nc.sync.dma_start(
                                oT_d[ec * 128:(ec + 1) * 128, ssl], ob[:, :]
                            )
                        return thunk
                    return [mk(ec) for ec in range(ECN)]

                def emit_pair(cur, prev, zT2s, op_work):
                    """Scores+exp for head-pair cur=(sc, m) (rows 0-63 /
                    64-127 run concurrently), interleaved with AV of the
                    previous pair and pending out-projection groups.
                    prev = (sc, m, eT2) or None. Returns new prev."""
                    pz_e = pz_o = None
                    if prev is not None:
                        pz_e = psum_z.tile([Dh + 1, SCW], dt.float32, tag="z")
                        pz_o = psum_z.tile([Dh + 1, SCW], dt.float32, tag="z")
                    eT2 = None
                    if cur is not None:
                        sc, m = cur
                        ssl = slice(sc * SCW, (sc + 1) * SCW)
                        eT2 = work.tile([128, TC, 2, SCW], dt.bfloat16, tag="expT")
                    for t in range(TC):
                        if eT2 is not None:
                            pst = psum_sT.tile([128, 2 * SCW], dt.float32, tag="sT")
                            for j in range(2):
                                hoff = j * Dh
                                nc.tensor.matmul(
                                    pst[:, j * SCW:(j + 1) * SCW],
                                    kT2[hoff:hoff + Dh, m, t * TCW:(t + 1) * TCW],
                                    qT2[hoff:hoff + Dh, m, ssl],
                                    start=True, stop=True,
                                    tile_position=(hoff, 0),
                                )
                            nc.scalar.activation(
                                eT2[:, t, :, :], pst[:, :], AF.Exp, scale=0.125,
                            )
                        if prev is not None:
                            scp, mp_, eT2p = prev
                            nc.tensor.matmul(
                                pz_e[:, :], v_sb[:, t, 2 * mp_, :],
                                eT2p[:, t, 0, :],
                                start=(t == 0), stop=(t == TC - 1),
                            )
                            nc.tensor.matmul(
                                pz_o[:, :], v_sb[:, t, 2 * mp_ + 1, :],
                                eT2p[:, t, 1, :],
                                start=(t == 0), stop=(t == TC - 1),
                            )
                        if op_work and t % 2 == 1:
                            op_work.pop(0)()
                    if prev is not None:
                        scp, mp_, _ = prev
                        emit_norm(0, pz_e, zT2s[scp], mp_)
                        emit_norm(1, pz_o, zT2s[scp], mp_)
                    if cur is None:
                        return None
                    return (cur[0], cur[1], eT2)

                if parts < 2:
                    for sc in range(SC):
                        for m in range(MP):
                            proj_qk(wq, 0, qT2, m, sc)
                else:
                    pairs = [(sc, m) for sc in range(SC) for m in range(MP)]
                    zT2s = {}
                    prev = None
                    op_work = []
                    proj_qk(wq, 0, qT2, pairs[0][1], pairs[0][0])
                    for i, cur in enumerate(pairs):
                        sc, m = cur
                        if m == 0:
                            zT2s[sc] = zpool.tile(
                                [128, MP, SCW], dt.bfloat16, tag="zT",
                                name=f"zT_{sc}")
                        if i + 1 < len(pairs):
                            proj_qk(wq, 0, qT2, pairs[i + 1][1], pairs[i + 1][0])
                        was_prev = prev
                        prev = emit_pair(cur, prev, zT2s, op_work)
                        if was_prev is not None and was_prev[1] == MP - 1 \
                                and parts >= 3:
                            op_work.extend(outproj_thunks(was_prev[0],
                                                          zT2s[was_prev[0]]))
                    emit_pair(None, prev, zT2s, op_work)
                    for thunk in op_work:
                        thunk()
                    if parts >= 3:
                        for thunk in outproj_thunks(SC - 1, zT2s[SC - 1]):
                            thunk()

    nc.compile()
    names = {
        "xT": xT_d.name, "wq": wq_d.name, "wk": wk_d.name, "wv": wv_d.name,
        "wo": wo_d.name, "bq": bq_d.name, "bk": bk_d.name, "bv": bv_d.name,
        "oT": oT_d.name,
    }
    return nc, names


def get_program():
    global _PROG
    if _PROG is None:
        _PROG = _build_program()
    return _PROG


def make_in_maps(x, Wq, bq, Wk, bk, Wv, bv, Wo, names):
    """Host-side sharding: per-core input dict (bf16 casts + layout prep)."""
    in_maps = []
    for c in range(N_CORES):
        b, g = divmod(c, 2)
        hsl = slice(g * HPC, (g + 1) * HPC)
        xT_c = np.ascontiguousarray(x[b].T).astype(BF16)                 # [E, S]
        wq_c = np.ascontiguousarray(
            Wq[hsl].transpose(1, 0, 2).reshape(E, HPC * Dh)).astype(BF16)
        wk_c = np.ascontiguousarray(
            Wk[hsl].transpose(1, 0, 2).reshape(E, HPC * Dh)).astype(BF16)
        wv_c = np.ascontiguousarray(
            Wv[hsl].transpose(1, 0, 2).reshape(E, HPC * Dh)).astype(BF16)
        # Wo rows for this head group, packed [Dh, HPC, E] (head on free axis)
        wo_c = np.ascontiguousarray(
            Wo[g * HPC * Dh:(g + 1) * HPC * Dh].reshape(MP, 128, E)
            .transpose(1, 0, 2)).astype(BF16)
        bq_c = np.ascontiguousarray(bq[hsl].reshape(MP, 128).T).astype(np.float32)
        bk_c = np.ascontiguousarray(bk[hsl].reshape(MP, 128).T).astype(np.float32)
        bv_c = bv[hsl].reshape(1, HPC * Dh).astype(BF16)
        in_maps.append({
            names["xT"]: xT_c, names["wq"]: wq_c, names["wk"]: wk_c,
            names["wv"]: wv_c, names["wo"]: wo_c, names["bq"]: bq_c,
            names["bk"]: bk_c, names["bv"]: bv_c,
        })
    return in_maps


def combine_outputs(results, bo, names):
    """Host-side unshard: sum head-group partials, transpose, add bo."""
    out = np.empty((B, S, E), np.float32)
    for b in range(B):
        oT = results[2 * b][names["oT"]] + results[2 * b + 1][names["oT"]]
        out[b] = oT.T + bo
    return out


_RUNNER = None


def _make_runner(nc):
    """Cached jit callable running `nc` SPMD on 8 cores via PJRT/axon.
    Mirrors run_bass_via_pjrt but is built once and reused across calls."""
    import jax
    from jax.sharding import Mesh, PartitionSpec
    try:
        from jax.experimental.shard_map import shard_map
    except ImportError:
        from jax import shard_map
    import concourse.mybir as mybir
    from concourse import bass2jax

    bass2jax.install_neuronx_cc_hook()
    pid_name = nc.partition_id_tensor.name if nc.partition_id_tensor else None
    in_names, out_names, out_avals, out_shapes = [], [], [], []
    for alloc in nc.m.functions[0].allocations:
        if not isinstance(alloc, mybir.MemoryLocationSet):
            continue
        name = alloc.memorylocations[0].name
        if alloc.kind == "ExternalInput" and name != pid_name:
            in_names.append(name)
        elif alloc.kind == "ExternalOutput":
            shape = tuple(alloc.tensor_shape)
            dtype = mybir.dt.np(alloc.dtype)
            out_names.append(name)
            out_avals.append(jax.core.ShapedArray(shape, dtype))
            out_shapes.append((shape, dtype))
    n_params = len(in_names)
    all_names = list(in_names) + list(out_names) + ([pid_name] if pid_name else [])

    def _body(*args):
        operands = list(args)
        if pid_name is not None:
            operands.append(bass2jax.partition_id_tensor())
        return tuple(bass2jax._bass_exec_p.bind(
            *operands, out_avals=tuple(out_avals), in_names=tuple(all_names),
            out_names=tuple(out_names), lowering_input_output_aliases=(),
            sim_require_finite=True, sim_require_nnan=True, nc=nc))

    devices = jax.devices()[:N_CORES]
    mesh = Mesh(np.asarray(devices), ("core",))
    nio = n_params + len(out_names)
    sharded = jax.jit(
        shard_map(_body, mesh=mesh, in_specs=(PartitionSpec("core"),) * nio,
                  out_specs=(PartitionSpec("core"),) * len(out_names),
                  check_rep=False),
        donate_argnums=tuple(range(n_params, nio)), keep_unused=True)

    def run(in_maps):
        concat_in = [
            np.concatenate([np.asarray(m[nm]) for m in in_maps], axis=0)
            for nm in in_names]
        zeros = [np.zeros((N_CORES * s[0], *s[1:]), dty)
                 for s, dty in out_shapes]
        outs = sharded(*concat_in, *zeros)
        return [
            {name: np.asarray(outs[i]).reshape(N_CORES, *out_shapes[i][0])[c]
             for i, name in enumerate(out_names)}
            for c in range(N_CORES)]

    return run


def kernel(x, Wq, bq, Wk, bk, Wv, bv, Wo, bo):
    global _RUNNER
    nc, names = get_program()
    in_maps = make_in_maps(
        np.asarray(x), np.asarray(Wq), np.asarray(bq), np.asarray(Wk),
        np.asarray(bk), np.asarray(Wv), np.asarray(bv), np.asarray(Wo), names,
    )
    try:
        if _RUNNER is None:
            _RUNNER = _make_runner(nc)
        results = _RUNNER(in_maps)
    except Exception:
        from concourse.bass_utils import run_bass_kernel_spmd
        _RUNNER = None
        results = run_bass_kernel_spmd(
            nc, in_maps, core_ids=list(range(N_CORES))).results
    return combine_outputs(results, np.asarray(bo, np.float32), names)



# revision 9
# speedup vs baseline: 2.8857x; 2.8857x over previous
"""Trainium2 Bass kernel: 16-head MHA (B=4, S=2048, E=1024, Dh=64), 8 cores.

Sharding: core c handles batch b = c//2 and head-group g = c%2 (8 heads).
Each core computes its 8 heads' attention plus the partial output
projection in transposed layout oT[e, s]; the host sums the two
head-group partials per batch, transposes, and adds bo.

Per-core dataflow (all matmuls bf16 with fp32 PSUM accumulation):
  qT/kT[d, s]  = Wq/Wk.T @ xT          (per head-pair, d stacked 2x64)
  v[t, hd]     = xT.T @ Wv + ones.T@bv (natural layout, + ones col for colsum)
  scoresT[t,s] = kT.T @ qT   (row-tiled: 2 heads in rows 0-63 / 64-127,
                              co-executed in the PE array)
  expT         = exp(0.125 * scoresT)  (ScalarE, cast to bf16)
  zT_un[d,s],colsum[s] = v_aug.T @ expT  (M=65: row 64 = colsum)
  zT           = zT_un * bcast(1/colsum) (bcast via K=1 matmul)
  oT[e, s]    += Wo_h.T @ zT_h  (accumulated over the core's 8 heads)

Software pipeline: the ScalarE exp stream (~1.15 us per [128,1024] tile,
256 tiles) is the binding engine, so all projection work (kT, v, qT) is
produced just-in-time from a deadline-sorted filler queue inside the
attention slot loop, filling the PE slack under the exp rate instead of
running as a serial prefix. Slot order is AV -> fillers -> scores so the
scores matmul's PSUM-bank wait (on exp two slots back) is covered by
work whose inputs are long ready (PE executes its stream in order).
"""

import numpy as np
import ml_dtypes

B, S, E = 4, 2048, 1024
H, Dh = 16, 64
N_CORES = 8
HPC = 8          # heads per core
MP = 4           # head-pairs per core
SC, SCW = 4, 512  # s-chunks
TC, TCW = 16, 128  # t-chunks
KE = 8           # k-tiles over E
ECN = 8          # e-chunks of 128 (outT partition tiles)

BF16 = ml_dtypes.bfloat16

_PROG = None

# Rough PE-time estimates (ns) used only to pace the filler queue.
COST_K = 1800    # kT/qT unit: 8 accumulating matmuls + bias add
COST_V = 2000    # v unit: 9 matmuls + copy
COST_O = 950     # outproj unit: 4 matmuls + evict
BUD_EARLY = 750  # opportunistic filler budget per slot, production phase
BUD_LATE = 500   # ... steady state


def _build_program(repeats=None, timing=False, parts=3, ablate=()):
    """Emit the Bass/Tile program. Returns (nc, names_dict).

    repeats: if set, wrap the whole body in a For_i loop (for marginal
    per-iteration HW timing; not used by the graded kernel() path).
    timing: demote the real output to internal DRAM and expose a tiny
    dummy output instead, so timing calls don't pay output transfers.
    parts: 1 = projections only, 2 = + attention, 3 = full (default).
    ablate: timing-only probes; subset of {"exp","av","norm"} — skips
    those ops (numerics break; never used by the graded kernel() path).
    """
    from contextlib import ExitStack

    import concourse.mybir as mybir
    import concourse.tile as tile
    from concourse import bacc

    ablate = set(ablate)
    if "av" in ablate:
        ablate.add("norm")

    dt = mybir.dt
    AF = mybir.ActivationFunctionType
    OP = mybir.AluOpType

    nc = bacc.Bacc(None, target_bir_lowering=False, debug=False)
    with tile.TileContext(nc) as tc:
        with tc.tile_pool(name="dram", bufs=1, space="DRAM") as dram:
            xT_d = dram.tile([E, S], dt.bfloat16, kind="ExternalInput")
            wq_d = dram.tile([E, HPC * Dh], dt.bfloat16, kind="ExternalInput")
            wk_d = dram.tile([E, HPC * Dh], dt.bfloat16, kind="ExternalInput")
            wv_d = dram.tile([E, HPC * Dh], dt.bfloat16, kind="ExternalInput")
            wo_d = dram.tile([128, MP, E], dt.bfloat16, kind="ExternalInput")
            bq_d = dram.tile([128, MP], dt.float32, kind="ExternalInput")
            bk_d = dram.tile([128, MP], dt.float32, kind="ExternalInput")
            bv_d = dram.tile([1, HPC * Dh], dt.bfloat16, kind="ExternalInput")
            if timing:
                oT_d = dram.tile([E, S], dt.float32, kind="Internal")
                dummy_d = dram.tile([1, 4], dt.bfloat16, kind="ExternalOutput")
            else:
                oT_d = dram.tile([E, S], dt.float32, kind="ExternalOutput")
                dummy_d = None

            with (
                tc.tile_pool(name="const", bufs=1) as const,
                tc.tile_pool(name="work", bufs=2) as work,
                tc.tile_pool(name="norm1", bufs=2) as norm1,
                tc.tile_pool(name="zpool", bufs=1) as zpool,
                tc.tile_pool(name="psum_sT", bufs=2, space="PSUM") as psum_sT,
                tc.tile_pool(name="psum_av", bufs=2, space="PSUM") as psum_av,
                tc.tile_pool(name="psum_z", bufs=2, space="PSUM") as psum_z,
                ExitStack() as _es,
            ):
                if repeats is not None:
                    _es.enter_context(tc.For_i(
                        0, repeats, 1,
                        hint_engines=(
                            mybir.EngineType.PE, mybir.EngineType.Activation,
                            mybir.EngineType.DVE, mybir.EngineType.SP,
                            mybir.EngineType.Pool,
                        ),
                    ))
                # ---- persistent SBUF ----
                xT = const.tile([128, KE, S], dt.bfloat16)
                wq = const.tile([128, KE, HPC * Dh], dt.bfloat16)
                wk = const.tile([128, KE, HPC * Dh], dt.bfloat16)
                wv = const.tile([128, KE, HPC * Dh], dt.bfloat16)
                wo = const.tile([128, MP, E], dt.bfloat16)
                bqk = const.tile([128, 2 * MP], dt.float32)
                onesbv = const.tile([1, HPC * Dh + 128], dt.bfloat16)
                qT2 = const.tile([128, MP, S], dt.bfloat16)
                kT2 = const.tile([128, MP, S], dt.bfloat16)
                v_sb = const.tile([128, TC, HPC, Dh + 1], dt.bfloat16)
                edummy = zdummy = None
                if "exp" in ablate:
                    edummy = const.tile([128, 2, SCW], dt.bfloat16)
                    nc.vector.memset(edummy[:, :, :], 0.001)
                if "norm" in ablate:
                    zdummy = const.tile([128, MP, SCW], dt.bfloat16)
                    nc.vector.memset(zdummy[:, :, :], 0.001)

                # ---- input DMAs, staged so the pipeline prefix is short:
                # wk+wq+x chunk 0 arrive first (needed by the prefix units),
                # the rest streams in underneath the early attention slots.
                xTr = xT_d[:].rearrange("(a p) c -> p a c", p=128)
                nc.sync.dma_start(
                    wk[:, :, :], wk_d[:].rearrange("(a p) c -> p a c", p=128))
                nc.sync.dma_start(bqk[:, 0:MP], bq_d[:])
                nc.sync.dma_start(bqk[:, MP:2 * MP], bk_d[:])
                nc.sync.dma_start(
                    wq[:, :, :], wq_d[:].rearrange("(a p) c -> p a c", p=128))
                for c in range(SC):
                    csl = slice(c * SCW, (c + 1) * SCW)
                    for k in range(KE):
                        nc.sync.dma_start(xT[:, k:k + 1, csl],
                                          xTr[:, k:k + 1, csl])
                    if c == 0:
                        nc.sync.dma_start(onesbv[0:1, 0:HPC * Dh], bv_d[:])
                        nc.vector.memset(onesbv[0:1, HPC * Dh:], 1.0)
                        nc.vector.memset(v_sb[:, :, :, Dh:Dh + 1], 1.0)
                        nc.sync.dma_start(
                            wv[:, :, :],
                            wv_d[:].rearrange("(a p) c -> p a c", p=128))
                nc.sync.dma_start(wo[:, :, :], wo_d[:])
                if dummy_d is not None:
                    nc.sync.dma_start(dummy_d[:, :], onesbv[0:1, 0:4])

                pairs = [(sc, m) for sc in range(SC) for m in range(MP)]

                # ---- production units (JIT-filled into attention slots) --
                def unit_proj_qk(w_sb, boff, dst, m, c):
                    def fn():
                        ssl = slice(c * SCW, (c + 1) * SCW)
                        p = psum_z.tile([128, SCW], dt.float32, tag="z")
                        for k in range(KE):
                            nc.tensor.matmul(
                                p[:, :],
                                w_sb[:, k, m * 128:(m + 1) * 128],
                                xT[:, k, ssl],
                                start=(k == 0), stop=(k == KE - 1),
                            )
                        nc.vector.tensor_scalar_add(
                            dst[:, m, ssl], p[:, :],
                            bqk[:, boff + m:boff + m + 1])
                    return fn

                def unit_v(t):
                    def fn():
                        tsl = slice(t * TCW, (t + 1) * TCW)
                        p = psum_z.tile([128, HPC * Dh], dt.float32, tag="z")
                        for k in range(KE):
                            nc.tensor.matmul(
                                p[:, :], xT[:, k, tsl], wv[:, k, :],
                                start=(k == 0), stop=False,
                            )
                        nc.tensor.matmul(
                            p[:, :], onesbv[0:1, HPC * Dh:HPC * Dh + 128],
                            onesbv[0:1, 0:HPC * Dh], start=False, stop=True,
                        )
                        nc.vector.tensor_copy(
                            v_sb[:, t, :, 0:Dh],
                            p[:, :].rearrange("p (h c) -> p h c", c=Dh),
                        )
                    return fn

                def unit_outproj(sc, ec, zT2):
                    def fn():
                        ssl = slice(sc * SCW, (sc + 1) * SCW)
                        po = psum_z.tile([128, SCW], dt.float32, tag="z",
                                         name=f"po_{sc}_{ec}")
                        for m in range(MP):
                            nc.tensor.matmul(
                                po[:, :],
                                wo[:, m, ec * 128:(ec + 1) * 128],
                                zT2[:, m, :],
                                start=(m == 0), stop=(m == MP - 1),
                            )
                        ob = work.tile([128, SCW], dt.float32, tag="ob",
                                       name=f"ob_{sc}_{ec}")
                        nc.vector.tensor_copy(ob[:, :], po[:, :])
                        nc.sync.dma_start(
                            oT_d[ec * 128:(ec + 1) * 128, ssl], ob[:, :])
                    return fn

                # Deadline-sorted filler queue. Deadline (pair, slot) = last
                # moment the unit may start (its consumer's slot); forcing at
                # each slot start keeps the PE stream topologically correct.
                Q = []

                def qforce(now):
                    while Q and Q[0][0] <= now:
                        Q.pop(0)[2]()

                def qpop_budget(budget):
                    while Q and budget >= Q[0][1]:
                        d, cst, fn = Q.pop(0)
                        fn()
                        budget -= cst

                for c in range(1, SC):
                    Q.append(((0, 4 * c), COST_K,
                              unit_proj_qk(wk, MP, kT2, 0, c)))
                for i in range(1, len(pairs)):
                    sci, mi = pairs[i]
                    Q.append(((i, 0), COST_K,
                              unit_proj_qk(wq, 0, qT2, mi, sci)))
                    if i < MP:
                        for c in range(SC):
                            Q.append(((i, 4 * c), COST_K,
                                      unit_proj_qk(wk, MP, kT2, i, c)))
                for t in range(TC):
                    Q.append(((1, t), COST_V, unit_v(t)))
                Q.sort(key=lambda e: e[0])

                # ---- attention helpers ----
                def emit_norm(hh, pz, zT2, m):
                    """Normalize head (pair m, parity hh) into zT2[:, m].
                    Odd heads land on partitions 64-127 via a shift DMA."""
                    cs = norm1.tile([1, SCW], dt.bfloat16, tag="cs")
                    nc.vector.tensor_copy(cs[0:1, :], pz[Dh:Dh + 1, :])
                    pbc = psum_z.tile([Dh, SCW], dt.float32, tag="z")
                    nc.tensor.matmul(
                        pbc[:, :], onesbv[0:1, HPC * Dh:HPC * Dh + Dh],
                        cs[0:1, :], start=True, stop=True,
                    )
                    bch = norm1.tile([Dh, SCW], dt.float32, tag="bch")
                    nc.vector.reciprocal(bch[:, :], pbc[:, :])
                    if hh == 0:
                        nc.vector.tensor_tensor(
                            zT2[0:Dh, m, :], pz[0:Dh, :], bch[:, :], OP.mult)
                    else:
                        ztmp = work.tile([Dh, SCW], dt.bfloat16, tag="ztmp")
                        nc.vector.tensor_tensor(
                            ztmp[:, :], pz[0:Dh, :], bch[:, :], OP.mult)
                        nc.sync.dma_start(zT2[Dh:2 * Dh, m, :], ztmp[:, :])

                def emit_pair(i, cur, prev, zT2s, bud):
                    """Slots for pair cur=(sc, m): AV of the previous pair,
                    filler pops, then scores+exp (last, so its PSUM-bank
                    wait on exp(t-2) is covered). prev = (sc, m, eT2)|None."""
                    do_av = prev is not None and "av" not in ablate
                    pz_e = pz_o = None
                    if do_av:
                        pz_e = psum_av.tile([Dh + 1, SCW], dt.float32, tag="av")
                        pz_o = psum_av.tile([Dh + 1, SCW], dt.float32, tag="av")
                    eT2 = None
                    if cur is not None:
                        sc, m = cur
                        ssl = slice(sc * SCW, (sc + 1) * SCW)
                        if "exp" not in ablate:
                            eT2 = work.tile(
                                [128, TC, 2, SCW], dt.bfloat16, tag="expT")
                    for t in range(TC):
                        qforce((i, t))
                        if do_av:
                            scp, mp_, eT2p = prev
                            esrc = (eT2p[:, t] if eT2p is not None
                                    else edummy[:, :, :])
                            nc.tensor.matmul(
                                pz_e[:, :], v_sb[:, t, 2 * mp_, :],
                                esrc[:, 0, :],
                                start=(t == 0), stop=(t == TC - 1),
                            )
                            nc.tensor.matmul(
                                pz_o[:, :], v_sb[:, t, 2 * mp_ + 1, :],
                                esrc[:, 1, :],
                                start=(t == 0), stop=(t == TC - 1),
                            )
                        qpop_budget(bud)
                        if cur is not None:
                            pst = psum_sT.tile(
                                [128, 2 * SCW], dt.float32, tag="sT")
                            for j in range(2):
                                hoff = j * Dh
                                nc.tensor.matmul(
                                    pst[:, j * SCW:(j + 1) * SCW],
                                    kT2[hoff:hoff + Dh, m,
                                        t * TCW:(t + 1) * TCW],
                                    qT2[hoff:hoff + Dh, m, ssl],
                                    start=True, stop=True,
                                    tile_position=(hoff, 0),
                                )
                            if eT2 is not None:
                                nc.scalar.activation(
                                    eT2[:, t, :, :], pst[:, :], AF.Exp,
                                    scale=0.125,
                                )
                    if do_av and "norm" not in ablate:
                        scp, mp_, _ = prev
                        emit_norm(0, pz_e, zT2s[scp], mp_)
                        emit_norm(1, pz_o, zT2s[scp], mp_)
                    if cur is None:
                        return None
                    return (cur[0], cur[1], eT2)

                # ---- main pipeline ----
                # Prefix: just enough production for pair 0's first slots.
                unit_proj_qk(wk, MP, kT2, 0, 0)()
                unit_proj_qk(wq, 0, qT2, 0, 0)()

                if parts < 2:
                    qforce((999, 0))
                else:
                    zT2s = {}
                    prev = None
                    for i, cur in enumerate(pairs):
                        sc, m = cur
                        if m == 0:
                            zT2s[sc] = zdummy if "norm" in ablate else \
                                zpool.tile([128, MP, SCW], dt.bfloat16,
                                           tag=f"zT_{sc}", name=f"zT_{sc}")
                        was_prev = prev
                        bud = BUD_EARLY if i <= MP else BUD_LATE
                        prev = emit_pair(i, cur, prev, zT2s, bud)
                        if was_prev is not None and was_prev[1] == MP - 1 \
                                and parts >= 3 and "norm" not in ablate:
                            scp = was_prev[0]
                            for ec in range(ECN):
                                Q.append(((998, ec), COST_O,
                                          unit_outproj(scp, ec, zT2s[scp])))
                    prev = emit_pair(len(pairs), None, prev, zT2s, BUD_LATE)
                    if parts >= 3 and "norm" not in ablate:
                        for ec in range(ECN):
                            Q.append(((998, ec), COST_O,
                                      unit_outproj(SC - 1, ec,
                                                   zT2s[SC - 1])))
                    qforce((999, 0))

    nc.compile()
    names = {
        "xT": xT_d.name, "wq": wq_d.name, "wk": wk_d.name, "wv": wv_d.name,
        "wo": wo_d.name, "bq": bq_d.name, "bk": bk_d.name, "bv": bv_d.name,
        "oT": oT_d.name,
    }
    return nc, names


def get_program():
    global _PROG
    if _PROG is None:
        _PROG = _build_program()
    return _PROG


def make_in_maps(x, Wq, bq, Wk, bk, Wv, bv, Wo, names):
    """Host-side sharding: per-core input dict (bf16 casts + layout prep)."""
    in_maps = []
    for c in range(N_CORES):
        b, g = divmod(c, 2)
        hsl = slice(g * HPC, (g + 1) * HPC)
        xT_c = np.ascontiguousarray(x[b].T).astype(BF16)                 # [E, S]
        wq_c = np.ascontiguousarray(
            Wq[hsl].transpose(1, 0, 2).reshape(E, HPC * Dh)).astype(BF16)
        wk_c = np.ascontiguousarray(
            Wk[hsl].transpose(1, 0, 2).reshape(E, HPC * Dh)).astype(BF16)
        wv_c = np.ascontiguousarray(
            Wv[hsl].transpose(1, 0, 2).reshape(E, HPC * Dh)).astype(BF16)
        # Wo rows for this head group, packed [Dh, HPC, E] (head on free axis)
        wo_c = np.ascontiguousarray(
            Wo[g * HPC * Dh:(g + 1) * HPC * Dh].reshape(MP, 128, E)
            .transpose(1, 0, 2)).astype(BF16)
        bq_c = np.ascontiguousarray(bq[hsl].reshape(MP, 128).T).astype(np.float32)
        bk_c = np.ascontiguousarray(bk[hsl].reshape(MP, 128).T).astype(np.float32)
        bv_c = bv[hsl].reshape(1, HPC * Dh).astype(BF16)
        in_maps.append({
            names["xT"]: xT_c, names["wq"]: wq_c, names["wk"]: wk_c,
            names["wv"]: wv_c, names["wo"]: wo_c, names["bq"]: bq_c,
            names["bk"]: bk_c, names["bv"]: bv_c,
        })
    return in_maps


def combine_outputs(results, bo, names):
    """Host-side unshard: sum head-group partials, transpose, add bo."""
    out = np.empty((B, S, E), np.float32)
    for b in range(B):
        oT = results[2 * b][names["oT"]] + results[2 * b + 1][names["oT"]]
        out[b] = oT.T + bo
    return out


_RUNNER = None


def _make_runner(nc):
    """Cached jit callable running `nc` SPMD on 8 cores via PJRT/axon.
    Mirrors run_bass_via_pjrt but is built once and reused across calls."""
    import jax
    from jax.sharding import Mesh, PartitionSpec
    try:
        from jax.experimental.shard_map import shard_map
    except ImportError:
        from jax import shard_map
    import concourse.mybir as mybir
    from concourse import bass2jax

    bass2jax.install_neuronx_cc_hook()
    pid_name = nc.partition_id_tensor.name if nc.partition_id_tensor else None
    in_names, out_names, out_avals, out_shapes = [], [], [], []
    for alloc in nc.m.functions[0].allocations:
        if not isinstance(alloc, mybir.MemoryLocationSet):
            continue
        name = alloc.memorylocations[0].name
        if alloc.kind == "ExternalInput" and name != pid_name:
            in_names.append(name)
        elif alloc.kind == "ExternalOutput":
            shape = tuple(alloc.tensor_shape)
            dtype = mybir.dt.np(alloc.dtype)
            out_names.append(name)
            out_avals.append(jax.core.ShapedArray(shape, dtype))
            out_shapes.append((shape, dtype))
    n_params = len(in_names)
    all_names = list(in_names) + list(out_names) + ([pid_name] if pid_name else [])

    def _body(*args):
        operands = list(args)
        if pid_name is not None:
            operands.append(bass2jax.partition_id_tensor())
        return tuple(bass2jax._bass_exec_p.bind(
            *operands, out_avals=tuple(out_avals), in_names=tuple(all_names),
            out_names=tuple(out_names), lowering_input_output_aliases=(),
            sim_require_finite=True, sim_require_nnan=True, nc=nc))

    devices = jax.devices()[:N_CORES]
    mesh = Mesh(np.asarray(devices), ("core",))
    nio = n_params + len(out_names)
    sharded = jax.jit(
        shard_map(_body, mesh=mesh, in_specs=(PartitionSpec("core"),) * nio,
                  out_specs=(PartitionSpec("core"),) * len(out_names),
                  check_rep=False),
        donate_argnums=tuple(range(n_params, nio)), keep_unused=True)

    def run(in_maps):
        concat_in = [
            np.concatenate([np.asarray(m[nm]) for m in in_maps], axis=0)
            for nm in in_names]
        zeros = [np.zeros((N_CORES * s[0], *s[1:]), dty)
                 for s, dty in out_shapes]
        outs = sharded(*concat_in, *zeros)
        return [
            {name: np.asarray(outs[i]).reshape(N_CORES, *out_shapes[i][0])[c]
             for i, name in enumerate(out_names)}
            for c in range(N_CORES)]

    return run


def kernel(x, Wq, bq, Wk, bk, Wv, bv, Wo, bo):
    global _RUNNER
    nc, names = get_program()
    in_maps = make_in_maps(
        np.asarray(x), np.asarray(Wq), np.asarray(bq), np.asarray(Wk),
        np.asarray(bk), np.asarray(Wv), np.asarray(bv), np.asarray(Wo), names,
    )
    try:
        if _RUNNER is None:
            _RUNNER = _make_runner(nc)
        results = _RUNNER(in_maps)
    except Exception:
        from concourse.bass_utils import run_bass_kernel_spmd
        _RUNNER = None
        results = run_bass_kernel_spmd(
            nc, in_maps, core_ids=list(range(N_CORES))).results
    return combine_outputs(results, np.asarray(bo, np.float32), names)
